# revision 2
# baseline (speedup 1.0000x reference)
"""Causal self-attention (GPT-style block) on 8 Trainium2 NeuronCores.

Problem: x[4, 2048, 768], w_attn[2304, 768], b_attn[2304], w_proj[768, 768],
b_proj[768]; 12 heads of size 64; causal softmax attention; output [4, 2048, 768].

Sharding: batch x heads. core = 2*b + g handles batch b (of 4) and the 6 heads
g*6..g*6+5 (tensor parallel over heads). Each core:
  1. QKV projection for its head slice, producing Q^T/K^T in [r, t] layout and
     V in [t, r] layout (plus a fused ones column for softmax denominators).
     The QK bias rides on the ACT-engine PSUM->SBUF copy (Identity + bias AP),
     so no bias matmuls on the PE for Q/K.
  2. Flash-style causal attention per head. kv blocks are processed in PAIRS:
     two S^T matmuls land in one [128, 1024] PSUM tile (2 banks) and a single
     1024-wide exp runs on ACT. The 4 diagonal tiles are merged into two
     packed exps ([512|384] and [256|256]); the last diagonal tile is widened
     to 256 columns (fp32r matmuls run 4x slower below 256 output columns)
     and killed with a [zeros|tril] mask. O^T accumulation is pipelined two
     steps behind S/exp so the PE doesn't wait on ACT round trips; softmax
     normalization is deferred one head for the same reason.
  3. QKV-phase PSUM->SBUF copies run on ACT (idle during that phase), keeping
     DVE free for the attention-phase mask/normalize work.
  4. c_proj triples are interleaved between the heads of the NEXT q-block's
     attention so the single-buffer PSUM slot never stalls the PE; the last
     q-block's c_proj reuses the (then idle) 2-bank S slots and ACT copies.
  5. Host reassembles (sums core pairs) and adds b_proj.

All matmuls run as float32r (TF32-like, 1 cycle/row at N>=256) with fp32 PSUM
accumulation.
"""
import os
from collections import deque

import numpy as np

os.environ.setdefault("JAX_COMPILATION_CACHE_DIR", "/tmp/jaxcache")
os.environ.setdefault("JAX_PERSISTENT_CACHE_MIN_COMPILE_TIME_SECS", "0")
os.environ.setdefault("JAX_PERSISTENT_CACHE_MIN_ENTRY_SIZE_BYTES", "0")

import concourse.bass as bass
import concourse.bacc as bacc
import concourse.tile as tile
from concourse import mybir
from concourse.bass_utils import run_bass_kernel_spmd

B, T, C, H = 4, 2048, 768, 12
HS = 64          # head size
HL = 6           # heads per core
CL = HL * HS     # 384 local channels per core
NQ = 512         # q block width
NCH = T // NQ    # 4 chunks
NKB = T // 128   # 16 kv blocks
NCORES = 8
F32 = mybir.dt.float32
F32R = mybir.dt.float32r
EXP = mybir.ActivationFunctionType.Exp
IDENT = mybir.ActivationFunctionType.Identity


def build_bass(repeat=1):
    nc = bacc.Bacc(num_devices=NCORES)
    xT = nc.declare_dram_parameter("xT", [C, T], F32, isOutput=False)
    wqkT = nc.declare_dram_parameter("wqkT", [C, 2 * CL], F32, isOutput=False)
    wvT = nc.declare_dram_parameter("wvT", [C, CL], F32, isOutput=False)
    wpT = nc.declare_dram_parameter("wpT", [CL, C], F32, isOutput=False)
    bqk2 = nc.declare_dram_parameter("bqk2", [128, 6], F32, isOutput=False)
    trz = nc.declare_dram_parameter("trz", [128, 256], F32, isOutput=False)
    ones = nc.declare_dram_parameter("ones", [128, NQ], F32, isOutput=False)
    y_out = nc.declare_dram_parameter("y_out", [T, C], F32, isOutput=True)

    with tile.TileContext(nc) as tc:
        with (
            tc.tile_pool(name="const", bufs=1) as constp,
            tc.tile_pool(name="wpool", bufs=1) as wpool,
            tc.tile_pool(name="qkv", bufs=1) as qkvp,
            tc.tile_pool(name="xch", bufs=2) as xchp,
            tc.tile_pool(name="ptp", bufs=4) as ptp,
            tc.tile_pool(name="otsb", bufs=2) as otsp,
            tc.tile_pool(name="yev", bufs=2) as yevp,
            tc.tile_pool(name="small", bufs=3) as smallp,
            tc.tile_pool(name="ps_big", bufs=2, space="PSUM") as psb,
            tc.tile_pool(name="ps_ot", bufs=2, space="PSUM") as psot,
            tc.tile_pool(name="ps_yp", bufs=2, space="PSUM") as psyp,
        ):
            engs = [nc.sync, nc.gpsimd]

            # ---- weights + constants. Only the wqk halves needed by the
            # first QK row-blocks go before chunk-0's x DMA; everything else
            # (second wqk halves, wv, mask, V-ones, wp) is issued right after
            # it via issue_late_loads().
            wqk_sb = []
            for cb in range(6):
                wt = wpool.tile([128, 2 * CL], F32R, tag=f"wqk{cb}")
                wqk_sb.append(wt)
            for rb in range(2):
                for cb in range(6):
                    engs[cb % 2].dma_start(
                        out=wqk_sb[cb][:, rb * 128:(rb + 1) * 128],
                        in_=wqkT[cb * 128:(cb + 1) * 128,
                                 rb * 128:(rb + 1) * 128].bitcast(F32R))
            bqk_sb = constp.tile([128, 6], F32)
            nc.sync.dma_start(out=bqk_sb, in_=bqk2[:, :])
            ones_sb = constp.tile([1, NQ], F32R)
            nc.gpsimd.dma_start(out=ones_sb, in_=ones[0:1, :].bitcast(F32R))

            wv_sb = []
            wp_sb = []
            late = {"done": False}

            def issue_late_loads():
                if late["done"]:
                    return
                late["done"] = True
                for rb in range(2, 6):
                    for cb in range(6):
                        engs[(rb + cb) % 2].dma_start(
                            out=wqk_sb[cb][:, rb * 128:(rb + 1) * 128],
                            in_=wqkT[cb * 128:(cb + 1) * 128,
                                     rb * 128:(rb + 1) * 128].bitcast(F32R))
                for cb in range(6):
                    wt = wpool.tile([128, CL], F32R, tag=f"wv{cb}")
                    engs[(cb + 1) % 2].dma_start(
                        out=wt, in_=wvT[cb * 128:(cb + 1) * 128, :].bitcast(F32R))
                    wv_sb.append(wt)
                trz_t = constp.tile([128, 256], F32)
                nc.sync.dma_start(out=trz_t, in_=trz[:, :])
                late["trz"] = trz_t
                nc.gpsimd.dma_start(
                    out=late["V"][:, :, :, HS],
                    in_=ones[:, 0:NKB * HL].bitcast(F32R).rearrange(
                        "p (a b) -> p a b", b=HL))
                for cb in range(3):
                    wt = wpool.tile([128, C], F32R, tag=f"wp{cb}")
                    engs[(cb + 2) % 2].dma_start(
                        out=wt, in_=wpT[cb * 128:(cb + 1) * 128, :].bitcast(F32R))
                    wp_sb.append(wt)

            # persistent activations
            QT = [qkvp.tile([128, T], F32R, tag=f"qt{i}", name=f"qt{i}") for i in range(3)]
            KT = [qkvp.tile([128, T], F32R, tag=f"kt{i}", name=f"kt{i}") for i in range(3)]
            V = qkvp.tile([128, NKB, HL, HS + 1], F32R, tag="v")
            late["V"] = V

            xTr = xT[:, :].bitcast(F32R).rearrange("(cb p) t -> p cb t", p=128)

            for _rep in range(repeat):
                late["done"] = _rep > 0
                phase_body(nc, tc, xTr, wqk_sb, wv_sb, wp_sb, bqk_sb,
                           ones_sb, QT, KT, V, y_out, late, issue_late_loads,
                           xchp, ptp, otsp, yevp, smallp, psb, psot, psyp)
    nc.finalize()
    return nc


def phase_body(nc, tc, xTr, wqk_sb, wv_sb, wp_sb, bqk_sb,
               ones_sb, QT, KT, V, y_out, late, issue_late_loads,
               xchp, ptp, otsp, yevp, smallp, psb, psot, psyp):
    engs = [nc.sync, nc.gpsimd]
    mm = nc.tensor.matmul

    # ---- Phase A: QKV projection per t-chunk ----
    for tcn in range(NCH):
        xc = xchp.tile([128, 6, NQ], F32R, tag="xc")
        for cb in range(6):
            engs[(cb + tcn) % 2].dma_start(
                out=xc[:, cb, :],
                in_=xTr[:, cb, tcn * NQ:(tcn + 1) * NQ])
        if tcn == 0:
            issue_late_loads()
        # Q^T / K^T: [r, t] layout, 6 row-blocks (3 Q + 3 K); bias rides on
        # the ACT copy (Identity activation with per-partition bias).
        for rb in range(6):
            ps = psb.tile([128, 2 * NQ], F32, tag="big")
            for cb in range(6):
                mm(ps[:, 0:NQ], lhsT=wqk_sb[cb][:, rb * 128:(rb + 1) * 128],
                   rhs=xc[:, cb, :], start=(cb == 0), stop=(cb == 5))
            dst = QT[rb] if rb < 3 else KT[rb - 3]
            nc.scalar.activation(dst[:, tcn * NQ:(tcn + 1) * NQ], ps[:, 0:NQ],
                                 IDENT, bias=bqk_sb[:, rb:rb + 1])
        # V: [t, r] layout, 4 t-subblocks
        for tb in range(4):
            ti = tcn * 4 + tb
            psv = psb.tile([128, 2 * NQ], F32, tag="big")
            for cb in range(6):
                mm(psv[:, 0:CL], lhsT=xc[:, cb, tb * 128:(tb + 1) * 128],
                   rhs=wv_sb[cb], start=(cb == 0), stop=(cb == 5))
            nc.scalar.copy(
                V[:, ti, :, 0:HS],
                psv[:, 0:CL].rearrange("p (h d) -> p h d", d=HS))

    trz_sb = late["trz"]

    # ---- Phase B: attention per q-block; c_proj of block J-1 interleaved ----
    cproj_prev = []
    prev_ots = None
    prev_J = -1
    norm_prev = None

    def issue_cproj(ots_t, J0, ti_i, half, pool, tag, act_copy=False):
        ti = J0 * 4 + ti_i
        yps = pool.tile([128, 2 * NQ] if tag == "big" else [128, CL],
                        F32, tag=tag)
        dst = yps[:, 0:CL]
        for cb in range(3):
            mm(dst, lhsT=ots_t[cb][:, ti_i * 128:(ti_i + 1) * 128],
               rhs=wp_sb[cb][:, half * CL:(half + 1) * CL],
               start=(cb == 0), stop=(cb == 2))
        yt = yevp.tile([128, CL], F32, tag="yt")
        if act_copy:
            nc.scalar.copy(yt, dst)
        else:
            nc.vector.tensor_copy(yt, dst)
        engs[(ti_i + half) % 2].dma_start(
            out=y_out[ti * 128:(ti + 1) * 128, half * CL:(half + 1) * CL],
            in_=yt)

    def issue_norm():
        nonlocal norm_prev
        if norm_prev is None:
            return
        ot_p, kb_p, po_p, ots_p = norm_prev
        norm_prev = None
        # recip of denominator row, broadcast partition 0 -> 64 partitions
        # on the (idle) GPSIMD engine, then one DVE multiply
        rec = smallp.tile([1, NQ], F32R, tag="rec")
        with nc.allow_low_precision(reason="fp32r matmul operand"):
            nc.vector.reciprocal(rec, ot_p[HS:HS + 1, :])
        bcs = smallp.tile([HS, NQ], F32R, tag="bcs")
        nc.gpsimd.partition_broadcast(bcs, rec, channels=HS)
        nc.vector.tensor_mul(ots_p[kb_p][po_p:po_p + HS, :],
                             ot_p[0:HS, :], bcs)

    tri = trz_sb[:, 128:256]
    for J in range(NCH):
        qs = slice(J * NQ, (J + 1) * NQ)
        ots = [otsp.tile([128, NQ], F32R, tag=f"ots{cb}", name=f"ots{cb}")
               for cb in range(3)]
        for h in range(HL):
            kb, po = h // 2, (h % 2) * HS
            qt = QT[kb][po:po + HS, qs]
            kt = KT[kb]
            ot = psot.tile([HS + 1, NQ], F32, tag="ot")
            # steps: kv-block pairs, then two merged diagonal pairs. O mms
            # are deferred two steps so PE stays ahead of ACT's exp latency.
            pend = deque()
            first = True

            def flush_one():
                for o in pend.popleft():
                    mm(o.pop("out"), **o)

            nsteps = 2 * J + 2
            for s in range(nsteps):
                sp = psb.tile([128, 2 * NQ], F32, tag="big")
                pt = ptp.tile([128, 2 * NQ], F32R, tag="pt")
                if s < 2 * J:
                    t0, t1 = 2 * s, 2 * s + 1
                    mm(sp[:, 0:NQ],
                       lhsT=kt[po:po + HS, t0 * 128:(t0 + 1) * 128],
                       rhs=qt, start=True, stop=True)
                    mm(sp[:, NQ:2 * NQ],
                       lhsT=kt[po:po + HS, t1 * 128:(t1 + 1) * 128],
                       rhs=qt, start=True, stop=True)
                    nc.scalar.activation(pt, sp, EXP, scale=0.125)
                    batch = [
                        dict(out=ot, lhsT=V[:, t0, h, :], rhs=pt[:, 0:NQ],
                             start=first, stop=False),
                        dict(out=ot, lhsT=V[:, t1, h, :], rhs=pt[:, NQ:2 * NQ],
                             start=False, stop=False)]
                elif s == 2 * J:
                    # diagonal tiles d=0 (512 wide) and d=1 (384 wide, packed
                    # at column 512) in one exp of 896 columns
                    t0, t1 = 4 * J, 4 * J + 1
                    mm(sp[:, 0:NQ],
                       lhsT=kt[po:po + HS, t0 * 128:(t0 + 1) * 128],
                       rhs=qt, start=True, stop=True)
                    mm(sp[:, NQ:NQ + 384],
                       lhsT=kt[po:po + HS, t1 * 128:(t1 + 1) * 128],
                       rhs=QT[kb][po:po + HS, J * NQ + 128:(J + 1) * NQ],
                       start=True, stop=True)
                    nc.scalar.activation(pt[:, 0:NQ + 384], sp[:, 0:NQ + 384],
                                         EXP, scale=0.125)
                    nc.vector.tensor_mul(pt[:, 0:128], pt[:, 0:128], tri)
                    nc.vector.tensor_mul(pt[:, NQ:NQ + 128],
                                         pt[:, NQ:NQ + 128], tri)
                    batch = [
                        dict(out=ot, lhsT=V[:, t0, h, :], rhs=pt[:, 0:NQ],
                             start=first, stop=False),
                        dict(out=ot[:, 128:NQ], lhsT=V[:, t1, h, :],
                             rhs=pt[:, NQ:NQ + 384], start=False, stop=False)]
                else:
                    # diagonal tiles d=2 and d=3, both 256 wide (d=3 widened;
                    # its invalid left half is killed by the [zeros|tril] mask)
                    t0, t1 = 4 * J + 2, 4 * J + 3
                    q2 = QT[kb][po:po + HS, J * NQ + 256:(J + 1) * NQ]
                    mm(sp[:, 0:256],
                       lhsT=kt[po:po + HS, t0 * 128:(t0 + 1) * 128],
                       rhs=q2, start=True, stop=True)
                    mm(sp[:, 256:NQ],
                       lhsT=kt[po:po + HS, t1 * 128:(t1 + 1) * 128],
                       rhs=q2, start=True, stop=True)
                    nc.scalar.activation(pt[:, 0:NQ], sp[:, 0:NQ],
                                         EXP, scale=0.125)
                    nc.vector.tensor_mul(pt[:, 0:128], pt[:, 0:128], tri)
                    nc.vector.tensor_mul(pt[:, 256:NQ], pt[:, 256:NQ], trz_sb)
                    batch = [
                        dict(out=ot[:, 256:NQ], lhsT=V[:, t0, h, :],
                             rhs=pt[:, 0:256], start=False, stop=False),
                        dict(out=ot[:, 256:NQ], lhsT=V[:, t1, h, :],
                             rhs=pt[:, 256:NQ], start=False, stop=False)]
                first = False
                pend.append(batch)
                if len(pend) > 2:
                    flush_one()
                if s == 0:
                    # previous head's normalize hides behind this head's mms
                    issue_norm()
            flat = [o for batch in pend for o in batch]
            flat[-1]["stop"] = True
            pend.clear()
            for o in flat:
                mm(o.pop("out"), **o)
            norm_prev = (ot, kb, po, ots)
            # interleave up to 2 pending c_proj items of the previous q-block
            for _ in range(2):
                if cproj_prev:
                    ti_i, half = cproj_prev.pop(0)
                    issue_cproj(prev_ots, prev_J, ti_i, half, psyp, "yps")
        while cproj_prev:
            ti_i, half = cproj_prev.pop(0)
            issue_cproj(prev_ots, prev_J, ti_i, half, psyp, "yps")
        prev_ots, prev_J = ots, J
        cproj_prev = [(i, half) for i in range(4) for half in range(2)]
    # tail: last head's normalize, then c_proj of the last q-block using the
    # (now idle) 2-bank S slots and ACT copies (DVE is busy normalizing).
    issue_norm()
    for ti_i, half in cproj_prev:
        issue_cproj(prev_ots, prev_J, ti_i, half, psb, "big", act_copy=True)


def make_in_maps(x, w_attn, b_attn, w_proj):
    x = np.asarray(x, dtype=np.float32)
    w_attn = np.asarray(w_attn, dtype=np.float32)
    b_attn = np.asarray(b_attn, dtype=np.float32)
    w_proj = np.asarray(w_proj, dtype=np.float32)
    # [zeros | tril]: right half masks the 128x128 diagonal block (valid iff
    # kv <= q, kv on partitions, q on free dim); left half zeroes the widened
    # region of the last diagonal tile.
    trz = np.concatenate(
        [np.zeros((128, 128), dtype=np.float32),
         np.triu(np.ones((128, 128), dtype=np.float32))], axis=1)
    in_maps = []
    for core in range(NCORES):
        b, g = divmod(core, 2)
        sl = slice(g * CL, (g + 1) * CL)
        wq, wk, wv = (w_attn[i * C:(i + 1) * C][sl] for i in range(3))
        bq, bk, bv_ = (b_attn[i * C:(i + 1) * C][sl] for i in range(3))
        in_maps.append({
            "xT": np.ascontiguousarray(x[b].T),
            "wqkT": np.ascontiguousarray(np.concatenate([wq, wk], 0).T),
            "wvT": np.ascontiguousarray(wv.T),
            "wpT": np.ascontiguousarray(w_proj[:, sl].T),
            # per-partition bias for the Identity-activation copy: column rb
            # holds the biases of QK row-block rb (partition = channel)
            "bqk2": np.ascontiguousarray(
                np.concatenate([bq, bk]).reshape(6, 128).T),
            "trz": trz,
            "ones": np.ones((128, NQ), dtype=np.float32),
        })
    return in_maps


def assemble(results, b_proj):
    out = np.empty((B, T, C), dtype=np.float32)
    for b in range(B):
        out[b] = results[2 * b]["y_out"] + results[2 * b + 1]["y_out"]
    out += np.asarray(b_proj, dtype=np.float32)[None, None, :]
    return out


_CACHE = {}


def _get_nc():
    if "nc" not in _CACHE:
        _CACHE["nc"] = build_bass()
    return _CACHE["nc"]


def kernel(x, w_attn, b_attn, w_proj, b_proj):
    in_maps = make_in_maps(x, w_attn, b_attn, w_proj)
    res = run_bass_kernel_spmd(_get_nc(), in_maps, list(range(NCORES)))
    # V-bias folds into a constant output offset: softmax weights sum to 1,
    # so attention output gets +bv, and y gets +w_proj @ bv.
    b_eff = (np.asarray(b_proj, np.float32)
             + np.asarray(w_proj, np.float32) @ np.asarray(
                 b_attn, np.float32)[2 * C:3 * C])
    return assemble(res.results, b_eff)
